# revision 21
# baseline (speedup 1.0000x reference)
"""DIFSR attention kernel for Trainium2, 8 NeuronCores, data-parallel over batch.

Math (per batch b):
  S_h = (Xid Wq_id)(Xid Wk_id)^T*s + (Xc Wq_c)(Xc Wk_c)^T*s + (Xp Wq_p)(Xp Wk_p)^T*s
        + rel_time_h + mask_add                       (s = HD^-0.5, folded into Q scale/bias)
  A_h = softmax_k(S_h);  O_h = A_h V_h;  y = concat_h(O_h) Wo + bo

Device dataflow is fully "transposed-activation" so no on-chip transposes exist:
  - host pre-transposes inputs to xT [HID, L], rel_time to [k, q] layout (mask
    folded in as -30000), and pre-swizzles every tensor into the exact SBUF
    partition-major layout so all DMAs are linear,
  - projections produce QT/KT [d, q] directly (weights stationary),
  - scores are computed as S^T [k, q] (K stationary), two heads packed into the
    128-partition dim via tile_position row groups (contraction K=64 each),
  - softmax denominator comes free from the PV matmul via a ones column
    appended to V (PSUM row 64 = sum_k E^T[k, q]); V slots are padded to an
    80-element stride and the PV stationary window is 128 wide so weight
    slices stay 32B-aligned with fast-weight-load enabled,
  - exp uses a fixed shift (no row max): attn = E/D is shift-invariant,
  - PV consumes E^T directly producing O^T; out-proj consumes O^T producing y
    in natural layout for a contiguous store,
  - the V bias never exists on device: rows of A sum to 1 after normalization,
    so  A(V + 1 bv^T) Wo + bo = (A V) Wo + (bv Wo + bo)  and the host folds
    bv into the output bias.

Schedule: the PE issue stream is the bottleneck (fp16 streams one column per
cycle; at the sustained P0 clock a 512-wide matmul costs ~259 ns), so the
emission order keeps the PE queue free of head-of-line blocking:
  - batch-major pair loop (b outer) — batch 0's output projection overlaps
    batch 1's pair compute instead of serializing at the end,
  - each pair's six projection chains are emitted one pair AHEAD, interleaved
    between the current pair's score/softmax stages, so the PE has dense
    independent work while the DVE(add-rel) -> ACT(exp) chain runs,
  - each pair's last PV matmul + normalize evac are deferred into the next
    iteration; the normalize multiply runs on the otherwise-idle GpSimd,
  - the final pair normalizes on the DVE in qt chunks and the last batch's
    out-projection pipelines behind it.

Precision: fp16 operands with fp32 PSUM accumulation; score+rel add, exp and
1/D in fp32.  Measured absmax-relative error vs the fp32 reference ~5.5e-4.
"""

import numpy as np

B, L, HID, NH, HD = 16, 512, 1024, 16, 64
NCORES = 8
BPC = B // NCORES  # batches per core
SHIFT = 4.0        # exp(s - SHIFT): keeps E in fp16 range for this data regime
MASKVAL = -30000.0
KT = HID // 128    # 8 contraction tiles
NJ = NH // 2       # 8 head pairs
NIT = BPC * NJ     # 16 pair iterations, batch-major

_CACHE = {}


def build_bass():
    import concourse.bass as bass
    import concourse.mybir as mybir
    import concourse.tile as tile
    from concourse import bacc
    from contextlib import ExitStack

    f16 = mybir.dt.float16
    f32 = mybir.dt.float32
    AF = mybir.ActivationFunctionType

    nc = bacc.Bacc()

    # All inputs are host-preswizzled to partition-major layouts (dim holding
    # 128 comes first; the rest is contiguous per partition) for linear DMA.
    xt = nc.dram_tensor("xt", [4, BPC, 128, KT, L], f16, kind="ExternalInput")
    wqk = nc.dram_tensor("wqk", [NJ, 128, 6, KT, 128], f16, kind="ExternalInput")
    wv = nc.dram_tensor("wv", [128, KT, HID], f16, kind="ExternalInput")
    wo = nc.dram_tensor("wo", [128, KT, HID], f16, kind="ExternalInput")
    bqk = nc.dram_tensor("bqk", [128, 6, KT], f32, kind="ExternalInput")
    bo2 = nc.dram_tensor("bo2", [HID], f16, kind="ExternalInput")  # bv@Wo + bo
    relt = nc.dram_tensor("relt", [BPC, NH, 128, 4, L], f16, kind="ExternalInput")
    y = nc.dram_tensor("y", [BPC, L, HID], f32, kind="ExternalOutput")

    with tile.TileContext(nc) as tc, ExitStack() as ctx:
        persist = ctx.enter_context(tc.tile_pool(name="persist", bufs=1))
        wslices = ctx.enter_context(tc.tile_pool(name="wslices", bufs=2))
        qkt_p = ctx.enter_context(tc.tile_pool(name="qkt", bufs=12))
        rel_p = ctx.enter_context(tc.tile_pool(name="relp", bufs=5))
        e_p = ctx.enter_context(tc.tile_pool(name="ep", bufs=4))
        rc_p = ctx.enter_context(tc.tile_pool(name="rcp", bufs=2))
        osb_p = ctx.enter_context(tc.tile_pool(name="osb", bufs=2))
        bc_p = ctx.enter_context(tc.tile_pool(name="bcp", bufs=2))
        ysb_p = ctx.enter_context(tc.tile_pool(name="ysb", bufs=3))
        ps_big = ctx.enter_context(tc.tile_pool(name="psbig", bufs=2, space="PSUM"))
        ps_s = ctx.enter_context(tc.tile_pool(name="pss", bufs=4, space="PSUM"))
        ps_o = ctx.enter_context(tc.tile_pool(name="pso", bufs=2, space="PSUM"))

        # ---- resident tiles ----
        xt_all = persist.tile([128, 3, BPC, KT, L], f16, tag="xt_all")
        # wv and wo are never live at the same time (the V phase finishes long
        # before the out-projection starts): one buffer, reloaded mid-run
        wvo_sb = persist.tile([128, KT, HID], f16, tag="wvo_sb")
        bqk_sb = persist.tile([128, 6, KT], f32, tag="bqk_sb")
        bo2_sb = persist.tile([1, HID], f16, tag="bo2_sb")
        ones1 = persist.tile([1, 128], f16, tag="ones1")
        ones32 = persist.tile([1, 64], f32, tag="ones32")
        expb = persist.tile([128, 1], f32, tag="expb")
        v_aug = persist.tile([128, BPC, 4, 16 * 80 + 48], f16, tag="v_aug")
        ot_all = persist.tile([128, BPC, NJ, L], f16, tag="ot_all")

        nc.vector.memset(ones1[:], 1.0)
        nc.vector.memset(ones32[:], 1.0)
        nc.vector.memset(expb[:], -SHIFT)
        # zero v_aug's padding (slot cols 65..79 and the 48-col tail) so the
        # 128-wide PV stationary windows never read uninitialized memory
        nc.vector.memset(
            v_aug[:].rearrange("p b t n -> p (b t) n")[:, :, 0:1280]
            .rearrange("p t (h c) -> p t h c", c=80)[:, :, :, 65:80], 0.0)
        nc.vector.memset(
            v_aug[:].rearrange("p b t n -> p (b t) n")[:, :, 1280:1328], 0.0)
        for b in range(BPC):
            for qt in range(4):
                nc.vector.memset(
                    v_aug[:, b, qt, 0:1280]
                    .rearrange("p (h c) -> p h c", c=80)[:, :, 64:65], 1.0)

        wsl_tiles = []
        rel_tiles = {}

        def prefetch_wsl(it):
            t = wslices.tile([128, 6, KT, 128], f16, tag="wsl", name="wsl")
            nc.sync.dma_start(out=t[:], in_=wqk[it % NJ])
            wsl_tiles.append(t)

        def prefetch_rel(it):
            # on the GpSimd queue: a rel issue that blocks on a free pool
            # buffer must never sit in front of the sync queue's bc broadcasts
            b, j = it // NJ, it % NJ
            rel = []
            for h01 in range(2):
                rt = rel_p.tile([128, 4, 512], f16, tag="relp", name="rel")
                nc.gpsimd.dma_start(out=rt[:], in_=relt[b, 2 * j + h01])
                rel.append(rt)
            rel_tiles[it] = rel

        # ---- startup DMA, spread across engine queues so the serial
        # DGE-issue cost (~0.7us per dma_start per queue) parallelizes.
        # xt_v lives in per-(b, kt) tiles so each V matmul's weight-load waits
        # only on its own slice's DMA, not on the whole X_v transfer.
        xtv_pool = ctx.enter_context(tc.tile_pool(name="xtv", bufs=2 * KT))
        xt_v = {}
        for b in range(BPC):
            for kt in range(KT):
                xt_v[b, kt] = xtv_pool.tile([128, L], f16, tag="xtv", name="xtv")
        for kt in range(KT):
            eng = nc.sync if kt % 2 == 0 else nc.gpsimd
            eng.dma_start(out=xt_v[0, kt][:], in_=xt[3, 0, :, kt])
            if kt % 2 == 0:
                nc.scalar.dma_start(out=wvo_sb[:, kt:kt + 2], in_=wv[:, kt:kt + 2])
        nc.gpsimd.dma_start(out=bqk_sb[:], in_=bqk[:])
        nc.gpsimd.dma_start(out=bo2_sb[:], in_=bo2[None, :])
        # pair-0/1 needs: first two weight slices + all three b=0 sources
        prefetch_wsl(0)
        for src in range(3):
            nc.gpsimd.dma_start(out=xt_all[:, src, 0], in_=xt[src, 0])
        for kt in range(KT):
            eng = nc.sync if kt % 2 == 0 else nc.gpsimd
            eng.dma_start(out=xt_v[1, kt][:], in_=xt[3, 1, :, kt])
        prefetch_wsl(1)
        prefetch_rel(0)
        for src in range(3):
            nc.sync.dma_start(out=xt_all[:, src, 1], in_=xt[src, 1])
        prefetch_rel(1)

        def emit_v_block(b, qt, nh):
            v_aug_b = v_aug[:, b, :, 0:1280].rearrange("p t (h c) -> p t h c", c=80)
            ps = ps_big.tile([128, 512], f32, tag="psbig", name="psv")
            for kt in range(KT):
                nc.tensor.matmul(
                    ps[:],
                    lhsT=xt_v[b, kt][:, qt * 128:(qt + 1) * 128],
                    rhs=wvo_sb[:, kt, nh * 512:(nh + 1) * 512],
                    start=(kt == 0), stop=(kt == KT - 1),
                )
            nc.vector.tensor_copy(
                v_aug_b[:, qt, nh * 8:(nh + 1) * 8, 0:64],
                ps[:].rearrange("p (h d) -> p h d", d=64),
            )

        # batch 0's V blocks run up front; batch 1's are scheduled into the
        # first loop iterations (they are only needed from iteration 8 on)
        for qt in range(4):
            for nh in range(2):
                emit_v_block(0, qt, nh)

        # ---- emission helpers ----
        def emit_proj_chain(it, w6):
            """One of the six Q/K projections for pair iteration `it`."""
            b, j = it // NJ, it % NJ
            wsl = wsl_tiles[it]
            src = w6 // 2
            ps = ps_big.tile([128, 512], f32, tag="psbig", name="psp")
            for kt in range(KT):
                nc.tensor.matmul(
                    ps[:],
                    lhsT=wsl[:, w6, kt],
                    rhs=xt_all[:, src, b, kt],
                    start=(kt == 0), stop=(kt == KT - 1),
                )
            t = qkt_p.tile([128, 512], f16, tag="qkt", name="qkt")
            is_q = (w6 % 2 == 0)
            nc.scalar.activation(
                t[:], ps[:], AF.Identity,
                bias=bqk_sb[:, w6, j:j + 1],
                scale=(float(HD) ** -0.5 if is_q else 1.0),
            )
            return t

        def emit_scores(qk, pss, kts):
            # h-major: each head's 3-source accumulation chain is contiguous, so
            # the other row-group's LDWEIGHTS/matmuls overlap across the chains
            for h01 in range(2):
                sl = slice(64 * h01, 64 * (h01 + 1))
                for si in range(3):
                    nc.tensor.matmul(
                        pss[h01][:],
                        lhsT=qk[2 * si + 1][sl, kts * 128:(kts + 1) * 128],
                        rhs=qk[2 * si][sl, :],
                        start=(si == 0), stop=(si == 2),
                        tile_position=(64 * h01, 0),
                    )

        def emit_softmax(pss, rel, kts):
            es = []
            for h01 in range(2):
                nc.vector.tensor_add(pss[h01][:], pss[h01][:], rel[h01][:, kts])
                e = e_p.tile([128, 512], f16, tag="ep", name="e")
                nc.scalar.activation(e[:], pss[h01][:], AF.Exp, bias=expb[:])
                es.append(e)
            return es

        def emit_pv(po, es, j, b, kts):
            # lhsT is a 128-wide window starting at the head's V slot: cols 0-63
            # are V, col 64 the ones column, the rest padding/next-slot data that
            # lands in PSUM rows 65-127 which are never read.  The full-width
            # stationary operand keeps fast-weight-load enabled.
            for h01 in range(2):
                base = (2 * j + h01) * 80
                nc.tensor.matmul(
                    po[h01][:],
                    lhsT=v_aug[:, b, kts, base:base + 128],
                    rhs=es[h01][:],
                    start=(kts == 0), stop=(kts == 3),
                )

        def emit_norm_head(po):
            # Evacuate [O_unnorm | D] to SBUF right away (frees the PSUM bank
            # for the next pair's PV accumulation — on the DVE, keeping the ACT
            # queue free for the exp chain), compute 1/D (fast seed+Newton on
            # DVE; the custom op needs a partition-0 SBUF operand) and launch
            # the partition-broadcast SBUF->SBUF DMA.  The final multiply is
            # emitted later so a slow broadcast can never block the FIFOs.
            out = []
            for h01 in range(2):
                osb = osb_p.tile([65, 512], f32, tag="osb", name="osb")
                nc.vector.tensor_copy(osb[:], po[h01][0:65, :])
                dsb = rc_p.tile([1, 512], f32, tag="dsb", name="dsb")
                nc.vector.tensor_copy(dsb[:], po[h01][64:65, :])
                rc = rc_p.tile([1, 512], f32, tag="rcp", name="rc")
                nc.vector.reciprocal_approx_fast(rc[:], dsb[:])
                bc = bc_p.tile([64, 512], f32, tag="bcp", name="bc")
                nc.sync.dma_start(
                    out=bc[:], in_=rc[0:1, None, :].broadcast_to([1, 64, 512])
                )
                out.append((osb, bc))
            return out

        def emit_norm_tail(po):
            # Final-pair variant: the 1/D broadcast runs as a tiny fp32 matmul
            # (ones column x rc) into the PSUM banks the PV just vacated — no
            # sync-queue DMA, ~4x lower latency on the kernel's critical tail.
            out = []
            for h01 in range(2):
                osb = osb_p.tile([65, 512], f32, tag="osb", name="osb")
                nc.vector.tensor_copy(osb[:], po[h01][0:65, :])
                dsb = rc_p.tile([1, 512], f32, tag="dsb", name="dsb")
                nc.vector.tensor_copy(dsb[:], po[h01][64:65, :])
                rc = rc_p.tile([1, 512], f32, tag="rcp", name="rc")
                nc.vector.reciprocal_approx_fast(rc[:], dsb[:])
                bcp = ps_o.tile([128, 512], f32, tag="pso", name="bcp")
                nc.tensor.matmul(
                    bcp[0:64, :], lhsT=ones32[:], rhs=rc[:], start=True, stop=True,
                )
                out.append((osb, bcp))
            return out

        def emit_norm_mul_gp(norm, j, b):
            # On GpSimd (otherwise idle): slower per element than DVE, but fully
            # off the DVE/ACT FIFOs, so the broadcast's DMA-queue latency is
            # harmless — nothing else waits on this engine.
            for h01, (osb, bc) in enumerate(norm):
                nc.gpsimd.tensor_mul(
                    ot_all[64 * h01:64 * (h01 + 1), b, j, :],
                    osb[0:64, :],
                    bc[:],
                )

        def emit_norm_mul_dve(norm, j, b):
            # Tail variant: DVE is idle by the last pair; qt-chunked so the
            # first out-projection tiles can start before the full multiply.
            for qt in range(4):
                qsl = slice(qt * 128, (qt + 1) * 128)
                for h01, (osb, bcp) in enumerate(norm):
                    nc.vector.tensor_mul(
                        ot_all[64 * h01:64 * (h01 + 1), b, j, qsl],
                        osb[0:64, qsl],
                        bcp[0:64, qsl],
                    )

        def emit_out_tile(b, qt, nh, pool=None, partial=None, store=None):
            """Output projection tile y[b, qt*128:, nh*512:].

            partial=(ps, lo, hi, finish): continue/finish a held accumulation
            instead of running all 8 pairs at once."""
            if partial is None:
                ps = (pool or ps_big).tile([128, 512], f32, tag=(pool or ps_big).name, name="psy")
                jlo, jhi, finish = 0, NJ, True
            else:
                ps, jlo, jhi, finish = partial
            for jj in range(jlo, jhi):
                nc.tensor.matmul(
                    ps[:],
                    lhsT=ot_all[:, b, jj, qt * 128:(qt + 1) * 128],
                    rhs=wvo_sb[:, jj, nh * 512:(nh + 1) * 512],
                    start=(jj == 0), stop=False,
                )
            if not finish:
                return ps
            nc.tensor.matmul(
                ps[:], lhsT=ones1[:], rhs=bo2_sb[:, nh * 512:(nh + 1) * 512],
                start=False, stop=True,
            )
            ysb = ysb_p.tile([128, 512], f32, tag="ysb", name="ysb")
            nc.vector.tensor_copy(ysb[:], ps[:])
            (store or nc.sync).dma_start(
                out=y[b, qt * 128:(qt + 1) * 128, nh * 512:(nh + 1) * 512],
                in_=ysb[:],
            )
            return None

        # ---- pair 0 projections run right after batch 0's V blocks (the loop
        # emits projections one pair ahead) ----
        qk_tiles = {0: [emit_proj_chain(0, w6) for w6 in range(6)]}

        # per-iteration filler PE work: batch 1's V blocks ride the early
        # iterations, batch 0's out-proj tiles the late ones (batch-major
        # order makes both overlap the pair pipeline instead of serializing)
        extra_sched = {i: [("v", 1, i, nh) for nh in range(2)] for i in range(4)}
        for i in range(7):
            extra_sched[8 + i] = [("o", 0, i // 2, i % 2)]

        pending = None      # (po, es3, j, b) — deferred last-PV + normalize
        mul_pending = None  # (norm, j, b) — deferred GpSimd multiply
        for t in range(NIT):
            b, j = t // NJ, t % NJ
            last = (t == NIT - 1)

            rel = rel_tiles.pop(t)
            qk = qk_tiles.pop(t)
            qk_next = []
            if not last:
                qk_tiles[t + 1] = qk_next
                # 3 projection chains ahead of the score pipeline; the other 3
                # are interleaved between score stages so the PE always has
                # dense independent work while DVE/ACT chew on the softmax.
                for w6 in range(3):
                    qk_next.append(emit_proj_chain(t + 1, w6))

            # kts=0 scores go before the deferred finish: the softmax chain
            # (DVE add -> ACT exp) starts as early as possible
            pss = [ps_s.tile([128, 512], f32, tag="pss", name="pss") for _ in range(2)]
            emit_scores(qk, pss, 0)
            es_prev = emit_softmax(pss, rel, 0)

            # deferred finish of pair t-1: last PV matmul + normalize evac
            if pending is not None:
                ppo, pes, pj, pb = pending
                emit_pv(ppo, pes, pj, pb, 3)
                mul_pending = (emit_norm_head(ppo), pj, pb)
                pending = None

            po = [ps_o.tile([128, 512], f32, tag="pso", name="po") for _ in range(2)]

            if not last:
                for kts in range(1, 4):
                    qk_next.append(emit_proj_chain(t + 1, 2 + kts))
                    pss = [ps_s.tile([128, 512], f32, tag="pss", name="pss") for _ in range(2)]
                    emit_scores(qk, pss, kts)
                    es = emit_softmax(pss, rel, kts)
                    emit_pv(po, es_prev, j, b, kts - 1)
                    es_prev = es
                    if kts == 2 and mul_pending is not None:
                        emit_norm_mul_gp(*mul_pending)
                        mul_pending = None
                pending = (po, es_prev, j, b)
                for task in extra_sched.get(t, ()):
                    if task[0] == "v":
                        emit_v_block(*task[1:])
                    else:
                        emit_out_tile(*task[1:])
                # end-of-body prefetches: anything that might block on a pool
                # buffer must sit behind this iteration's bc broadcasts/stores
                # in its queue, never in front of them
                if t + 2 < NIT:
                    prefetch_rel(t + 2)
                    prefetch_wsl(t + 2)
                if t == 3:
                    nc.sync.dma_start(out=wvo_sb[:], in_=wo[:])
            else:
                # final iteration: no next-pair projections; fill the
                # interleave slots with batch-0's last out tile and batch-1
                # qt=0 partials, then finish this pair inline with the
                # low-latency matmul-broadcast normalize and pipeline the b1
                # out-projection behind it.
                if mul_pending is not None:
                    emit_norm_mul_gp(*mul_pending)
                    mul_pending = None
                emit_out_tile(0, 3, 1)
                part = [None, None]
                for kts in range(1, 4):
                    pss = [ps_s.tile([128, 512], f32, tag="pss", name="pss") for _ in range(2)]
                    emit_scores(qk, pss, kts)
                    es = emit_softmax(pss, rel, kts)
                    emit_pv(po, es_prev, j, b, kts - 1)
                    es_prev = es
                    if kts in (1, 2):
                        nh = kts - 1
                        ps = ps_big.tile([128, 512], f32, tag="psbig", name="psy")
                        part[nh] = emit_out_tile(1, 0, nh, partial=(ps, 0, NJ - 1, False))
                emit_pv(po, es_prev, j, b, 3)
                norm = emit_norm_tail(po)
                emit_norm_mul_dve(norm, j, b)
                # qt=0 tiles: only the last pair's contraction remains; stores
                # go out on the idle GpSimd queue so the sync queue's serial
                # issue cost never tails the kernel.
                for nh in range(2):
                    emit_out_tile(1, 0, nh, partial=(part[nh], NJ - 1, NJ, True),
                                  store=nc.gpsimd)
                for qt in range(1, 4):
                    for nh in range(2):
                        pool = ps_big if nh == 0 else ps_s
                        emit_out_tile(1, qt, nh, pool=pool, store=nc.gpsimd)

    nc.finalize()
    return nc


def prep_inputs(inputs):
    """Host-side sharding + layout prep. Returns per-core in_maps.

    Every device tensor is laid out partition-major so DMAs are linear:
    the value at SBUF (partition p, ...) sits contiguously in DRAM.
    """
    f16 = np.float16
    inputs = {k: np.asarray(v) for k, v in inputs.items()}
    s = float(HD) ** -0.5

    # xt: [4, B, 128p, KT, L] where (kt*128+p) indexes HID of x^T [HID, L]
    xt_full = np.empty((4, B, 128, KT, L), f16)
    for i, k in enumerate(("seq_id", "seq_cate", "seq_pos", "V_id_input")):
        x = inputs[k].astype(f16)                       # [B, L, HID]
        xt = x.transpose(0, 2, 1)                       # [B, HID, L]
        xt_full[i] = xt.reshape(B, KT, 128, L).transpose(0, 2, 1, 3)

    # wqk: [NJ, 128p, 6, KT, 128n] — per head-pair column slices of the six
    # Q/K weight matrices, hid_in = kt*128+p.
    wqk_st = np.stack(
        [inputs[k] for k in ("q_id_w", "k_id_w", "q_cate_w", "k_cate_w", "q_pos_w", "k_pos_w")]
    ).astype(f16)                                       # [6, HID, HID]
    wqk_r = wqk_st.reshape(6, KT, 128, NJ, 128)          # [6, kt, p, j, n]
    wqk_lin = np.ascontiguousarray(wqk_r.transpose(3, 2, 0, 1, 4))  # [j, p, 6, kt, n]

    def w_lin(w):  # [HID, HID] -> [128p, KT, HID]
        return np.ascontiguousarray(
            w.astype(f16).reshape(KT, 128, HID).transpose(1, 0, 2)
        )

    wv_lin = w_lin(inputs["v_id_w"])
    wo_lin = w_lin(inputs["out_w"])

    bqk_st = np.stack(
        [
            inputs["q_id_b"] * s, inputs["k_id_b"],
            inputs["q_cate_b"] * s, inputs["k_cate_b"],
            inputs["q_pos_b"] * s, inputs["k_pos_b"],
        ]
    ).astype(np.float32)                                # [6, HID]
    bqk_lin = np.ascontiguousarray(
        bqk_st.reshape(6, KT, 128).transpose(2, 0, 1)   # [128p, 6, kt]
    ).astype(np.float32)
    # rows of the normalized attention sum to 1, so the V bias collapses into
    # the output bias: y = (A V')Wo + (bv Wo + bo)
    bo2_h = (
        inputs["v_id_b"].astype(np.float64) @ inputs["out_w"].astype(np.float64)
        + inputs["out_b"].astype(np.float64)
    ).astype(f16)

    # relt: [B, NH, 128p, 4kts, L] with (kts*128+p) indexing k of rel^T [k, q]
    relT = np.empty((B, NH, 128, 4, L), f16)
    for b in range(B):
        maskadd = np.where(inputs["attn_mask"][b], np.float32(0), np.float32(MASKVAL))
        relb = inputs["relative_time"][b].astype(np.float32) + maskadd[None]
        rT = relb.transpose(0, 2, 1).astype(f16)         # [NH, k, q]
        relT[b] = rT.reshape(NH, 4, 128, L).transpose(0, 2, 1, 3)

    in_maps = []
    for c in range(NCORES):
        bs = slice(c * BPC, (c + 1) * BPC)
        in_maps.append(
            {
                "xt": np.ascontiguousarray(xt_full[:, bs]),
                "wqk": wqk_lin, "wv": wv_lin, "wo": wo_lin,
                "bqk": bqk_lin, "bo2": bo2_h,
                "relt": np.ascontiguousarray(relT[bs]),
            }
        )
    return in_maps


def kernel(**inputs):
    from concourse.bass_utils import run_bass_kernel_spmd

    if "nc" not in _CACHE:
        _CACHE["nc"] = build_bass()
    nc = _CACHE["nc"]
    in_maps = prep_inputs(inputs)
    res = run_bass_kernel_spmd(nc, in_maps, list(range(NCORES)))
    out = np.concatenate([res.results[c]["y"] for c in range(NCORES)], axis=0)
    return out.astype(np.float32)


# revision 26
# speedup vs baseline: 1.0042x; 1.0042x over previous
"""DIFSR attention kernel for Trainium2, 8 NeuronCores, data-parallel over batch.

Math (per batch b):
  S_h = (Xid Wq_id)(Xid Wk_id)^T*s + (Xc Wq_c)(Xc Wk_c)^T*s + (Xp Wq_p)(Xp Wk_p)^T*s
        + rel_time_h + mask_add                       (s = HD^-0.5, folded into Q scale/bias)
  A_h = softmax_k(S_h);  O_h = A_h V_h;  y = concat_h(O_h) Wo + bo

Device dataflow is fully "transposed-activation" so no on-chip transposes exist:
  - host pre-transposes inputs to xT [HID, L], rel_time to [k, q] layout (mask
    folded in as -30000), and pre-swizzles every tensor into the exact SBUF
    partition-major layout so all DMAs are linear,
  - projections produce QT/KT [d, q] directly (weights stationary),
  - scores are computed as S^T [k, q] (K stationary), two heads packed into the
    128-partition dim via tile_position row groups (contraction K=64 each),
  - softmax denominator comes free from the PV matmul via a ones column
    appended to V (PSUM row 64 = sum_k E^T[k, q]); V slots are padded to an
    80-element stride and the PV stationary window is 128 wide so weight
    slices stay 32B-aligned with fast-weight-load enabled,
  - exp uses a fixed shift (no row max): attn = E/D is shift-invariant,
  - PV consumes E^T directly producing O^T; out-proj consumes O^T producing y
    in natural layout for a contiguous store,
  - the V bias never exists on device: rows of A sum to 1 after normalization,
    so  A(V + 1 bv^T) Wo + bo = (A V) Wo + (bv Wo + bo)  and the host folds
    bv into the output bias.

Schedule: the PE issue stream is the bottleneck (fp16 streams one column per
cycle; at the sustained P0 clock a 512-wide matmul costs ~259 ns), so the
emission order keeps the PE queue free of head-of-line blocking:
  - batch-major pair loop (b outer) — batch 0's output projection overlaps
    batch 1's pair compute instead of serializing at the end,
  - each pair's six projection chains are emitted one pair AHEAD, interleaved
    between the current pair's score/softmax stages, so the PE has dense
    independent work while the DVE(add-rel) -> ACT(exp) chain runs,
  - each pair's last PV matmul + normalize evac are deferred into the next
    iteration; the normalize multiply runs on the otherwise-idle GpSimd,
  - the final pair normalizes on the DVE in qt chunks and the last batch's
    out-projection pipelines behind it.

Precision: fp16 operands with fp32 PSUM accumulation; score+rel add, exp and
1/D in fp32.  Measured absmax-relative error vs the fp32 reference ~5.5e-4.
"""

import numpy as np

B, L, HID, NH, HD = 16, 512, 1024, 16, 64
NCORES = 8
BPC = B // NCORES  # batches per core
SHIFT = 4.0        # exp(s - SHIFT): keeps E in fp16 range for this data regime
MASKVAL = -30000.0
KT = HID // 128    # 8 contraction tiles
NJ = NH // 2       # 8 head pairs
NIT = BPC * NJ     # 16 pair iterations, batch-major

_CACHE = {}


def build_bass():
    import concourse.bass as bass
    import concourse.mybir as mybir
    import concourse.tile as tile
    from concourse import bacc
    from contextlib import ExitStack

    f16 = mybir.dt.float16
    f32 = mybir.dt.float32
    AF = mybir.ActivationFunctionType

    nc = bacc.Bacc()

    # All inputs are host-preswizzled to partition-major layouts (dim holding
    # 128 comes first; the rest is contiguous per partition) for linear DMA.
    xt = nc.dram_tensor("xt", [4, BPC, 128, KT, L], f16, kind="ExternalInput")
    wqk = nc.dram_tensor("wqk", [NJ, 128, 6, KT, 128], f16, kind="ExternalInput")
    wv = nc.dram_tensor("wv", [128, KT, HID], f16, kind="ExternalInput")
    wo = nc.dram_tensor("wo", [128, KT, HID], f16, kind="ExternalInput")
    bqk = nc.dram_tensor("bqk", [128, 6, KT], f32, kind="ExternalInput")
    bo2 = nc.dram_tensor("bo2", [HID], f16, kind="ExternalInput")  # bv@Wo + bo
    relt = nc.dram_tensor("relt", [BPC, NH, 128, 4, L], f16, kind="ExternalInput")
    y = nc.dram_tensor("y", [BPC, L, HID], f32, kind="ExternalOutput")

    with tile.TileContext(nc) as tc, ExitStack() as ctx:
        persist = ctx.enter_context(tc.tile_pool(name="persist", bufs=1))
        wslices = ctx.enter_context(tc.tile_pool(name="wslices", bufs=2))
        qkt_p = ctx.enter_context(tc.tile_pool(name="qkt", bufs=12))
        rel_p = ctx.enter_context(tc.tile_pool(name="relp", bufs=5))
        e_p = ctx.enter_context(tc.tile_pool(name="ep", bufs=4))
        rc_p = ctx.enter_context(tc.tile_pool(name="rcp", bufs=2))
        osb_p = ctx.enter_context(tc.tile_pool(name="osb", bufs=2))
        bc_p = ctx.enter_context(tc.tile_pool(name="bcp", bufs=2))
        ysb_p = ctx.enter_context(tc.tile_pool(name="ysb", bufs=3))
        ps_big = ctx.enter_context(tc.tile_pool(name="psbig", bufs=2, space="PSUM"))
        ps_s = ctx.enter_context(tc.tile_pool(name="pss", bufs=4, space="PSUM"))
        ps_o = ctx.enter_context(tc.tile_pool(name="pso", bufs=2, space="PSUM"))

        # ---- resident tiles ----
        xt_all = persist.tile([128, 3, BPC, KT, L], f16, tag="xt_all")
        # wv and wo are never live at the same time (the V phase finishes long
        # before the out-projection starts): one buffer, reloaded mid-run
        wvo_sb = persist.tile([128, KT, HID], f16, tag="wvo_sb")
        bqk_sb = persist.tile([128, 6, KT], f32, tag="bqk_sb")
        bo2_sb = persist.tile([1, HID], f16, tag="bo2_sb")
        ones1 = persist.tile([1, 128], f16, tag="ones1")
        ones32 = persist.tile([1, 64], f32, tag="ones32")
        expb = persist.tile([128, 1], f32, tag="expb")
        v_aug = persist.tile([128, BPC, 4, 16 * 80 + 48], f16, tag="v_aug")
        ot_all = persist.tile([128, BPC, NJ, L], f16, tag="ot_all")

        nc.vector.memset(ones1[:], 1.0)
        nc.vector.memset(ones32[:], 1.0)
        nc.vector.memset(expb[:], -SHIFT)
        # zero v_aug's padding (slot cols 65..79 and the 48-col tail) so the
        # 128-wide PV stationary windows never read uninitialized memory
        nc.vector.memset(
            v_aug[:].rearrange("p b t n -> p (b t) n")[:, :, 0:1280]
            .rearrange("p t (h c) -> p t h c", c=80)[:, :, :, 65:80], 0.0)
        nc.vector.memset(
            v_aug[:].rearrange("p b t n -> p (b t) n")[:, :, 1280:1328], 0.0)
        for b in range(BPC):
            for qt in range(4):
                nc.vector.memset(
                    v_aug[:, b, qt, 0:1280]
                    .rearrange("p (h c) -> p h c", c=80)[:, :, 64:65], 1.0)

        wsl_tiles = []
        rel_tiles = {}

        def prefetch_wsl(it):
            t = wslices.tile([128, 6, KT, 128], f16, tag="wsl", name="wsl")
            nc.sync.dma_start(out=t[:], in_=wqk[it % NJ])
            wsl_tiles.append(t)

        def prefetch_rel(it):
            # on the GpSimd queue: a rel issue that blocks on a free pool
            # buffer must never sit in front of the sync queue's bc broadcasts
            b, j = it // NJ, it % NJ
            rel = []
            for h01 in range(2):
                rt = rel_p.tile([128, 4, 512], f16, tag="relp", name="rel")
                nc.gpsimd.dma_start(out=rt[:], in_=relt[b, 2 * j + h01])
                rel.append(rt)
            rel_tiles[it] = rel

        # ---- startup DMA, spread across engine queues so the serial
        # DGE-issue cost (~0.7us per dma_start per queue) parallelizes.
        # xt_v lives in per-(b, kt) tiles so each V matmul's weight-load waits
        # only on its own slice's DMA, not on the whole X_v transfer.
        xtv_pool = ctx.enter_context(tc.tile_pool(name="xtv", bufs=2 * KT))
        xt_v = {}
        for b in range(BPC):
            for kt in range(KT):
                xt_v[b, kt] = xtv_pool.tile([128, L], f16, tag="xtv", name="xtv")
        for kt in range(KT):
            eng = nc.sync if kt % 2 == 0 else nc.gpsimd
            eng.dma_start(out=xt_v[0, kt][:], in_=xt[3, 0, :, kt])
            if kt % 2 == 0:
                nc.scalar.dma_start(out=wvo_sb[:, kt:kt + 2], in_=wv[:, kt:kt + 2])
        nc.gpsimd.dma_start(out=bqk_sb[:], in_=bqk[:])
        nc.gpsimd.dma_start(out=bo2_sb[:], in_=bo2[None, :])
        # pair-0/1 needs: first two weight slices + all three b=0 sources
        prefetch_wsl(0)
        for src in range(3):
            nc.gpsimd.dma_start(out=xt_all[:, src, 0], in_=xt[src, 0])
        for kt in range(KT):
            eng = nc.sync if kt % 2 == 0 else nc.gpsimd
            eng.dma_start(out=xt_v[1, kt][:], in_=xt[3, 1, :, kt])
        prefetch_wsl(1)
        prefetch_rel(0)
        for src in range(3):
            nc.sync.dma_start(out=xt_all[:, src, 1], in_=xt[src, 1])
        prefetch_rel(1)

        def emit_v_block(b, qt, nh):
            v_aug_b = v_aug[:, b, :, 0:1280].rearrange("p t (h c) -> p t h c", c=80)
            ps = ps_big.tile([128, 512], f32, tag="psbig", name="psv")
            for kt in range(KT):
                nc.tensor.matmul(
                    ps[:],
                    lhsT=xt_v[b, kt][:, qt * 128:(qt + 1) * 128],
                    rhs=wvo_sb[:, kt, nh * 512:(nh + 1) * 512],
                    start=(kt == 0), stop=(kt == KT - 1),
                )
            nc.vector.tensor_copy(
                v_aug_b[:, qt, nh * 8:(nh + 1) * 8, 0:64],
                ps[:].rearrange("p (h d) -> p h d", d=64),
            )

        # batch 0's V blocks run up front; batch 1's are scheduled into the
        # first loop iterations (they are only needed from iteration 8 on)
        for qt in range(4):
            for nh in range(2):
                emit_v_block(0, qt, nh)

        # ---- emission helpers ----
        def emit_proj_chain(it, w6):
            """One of the six Q/K projections for pair iteration `it`."""
            b, j = it // NJ, it % NJ
            wsl = wsl_tiles[it]
            src = w6 // 2
            ps = ps_big.tile([128, 512], f32, tag="psbig", name="psp")
            for kt in range(KT):
                nc.tensor.matmul(
                    ps[:],
                    lhsT=wsl[:, w6, kt],
                    rhs=xt_all[:, src, b, kt],
                    start=(kt == 0), stop=(kt == KT - 1),
                )
            t = qkt_p.tile([128, 512], f16, tag="qkt", name="qkt")
            is_q = (w6 % 2 == 0)
            nc.scalar.activation(
                t[:], ps[:], AF.Identity,
                bias=bqk_sb[:, w6, j:j + 1],
                scale=(float(HD) ** -0.5 if is_q else 1.0),
            )
            return t

        def emit_scores(qk, pss, kts):
            # h-major: each head's 3-source accumulation chain is contiguous, so
            # the other row-group's LDWEIGHTS/matmuls overlap across the chains
            for h01 in range(2):
                sl = slice(64 * h01, 64 * (h01 + 1))
                for si in range(3):
                    nc.tensor.matmul(
                        pss[h01][:],
                        lhsT=qk[2 * si + 1][sl, kts * 128:(kts + 1) * 128],
                        rhs=qk[2 * si][sl, :],
                        start=(si == 0), stop=(si == 2),
                        tile_position=(64 * h01, 0),
                    )

        def emit_softmax(pss, rel, kts):
            es = []
            for h01 in range(2):
                nc.vector.tensor_add(pss[h01][:], pss[h01][:], rel[h01][:, kts])
                e = e_p.tile([128, 512], f16, tag="ep", name="e")
                nc.scalar.activation(e[:], pss[h01][:], AF.Exp, bias=expb[:])
                es.append(e)
            return es

        def emit_pv(po, es, j, b, kts):
            # lhsT is a 128-wide window starting at the head's V slot: cols 0-63
            # are V, col 64 the ones column, the rest padding/next-slot data that
            # lands in PSUM rows 65-127 which are never read.  The full-width
            # stationary operand keeps fast-weight-load enabled.
            for h01 in range(2):
                base = (2 * j + h01) * 80
                nc.tensor.matmul(
                    po[h01][:],
                    lhsT=v_aug[:, b, kts, base:base + 128],
                    rhs=es[h01][:],
                    start=(kts == 0), stop=(kts == 3),
                )

        def emit_norm_head(po):
            # Evacuate [O_unnorm | D] to SBUF right away (frees the PSUM bank
            # for the next pair's PV accumulation — on the DVE, keeping the ACT
            # queue free for the exp chain), compute 1/D (fast seed+Newton on
            # DVE; the custom op needs a partition-0 SBUF operand) and launch
            # the partition-broadcast SBUF->SBUF DMA.  The final multiply is
            # emitted later so a slow broadcast can never block the FIFOs.
            out = []
            for h01 in range(2):
                osb = osb_p.tile([65, 512], f32, tag="osb", name="osb")
                nc.scalar.copy(osb[:], po[h01][0:65, :])
                dsb = rc_p.tile([1, 512], f32, tag="dsb", name="dsb")
                nc.scalar.copy(dsb[:], po[h01][64:65, :])
                rc = rc_p.tile([1, 512], f32, tag="rcp", name="rc")
                nc.vector.reciprocal_approx_fast(rc[:], dsb[:])
                bc = bc_p.tile([64, 512], f32, tag="bcp", name="bc")
                nc.sync.dma_start(
                    out=bc[:], in_=rc[0:1, None, :].broadcast_to([1, 64, 512])
                )
                out.append((osb, bc))
            return out

        def emit_norm_tail(po):
            # Final-pair variant: the 1/D broadcast runs as a tiny fp32 matmul
            # (ones column x rc) into the PSUM banks the PV just vacated — no
            # sync-queue DMA, ~4x lower latency on the kernel's critical tail.
            out = []
            for h01 in range(2):
                osb = osb_p.tile([65, 512], f32, tag="osb", name="osb")
                nc.vector.tensor_copy(osb[:], po[h01][0:65, :])
                dsb = rc_p.tile([1, 512], f32, tag="dsb", name="dsb")
                nc.vector.tensor_copy(dsb[:], po[h01][64:65, :])
                rc = rc_p.tile([1, 512], f32, tag="rcp", name="rc")
                nc.vector.reciprocal_approx_fast(rc[:], dsb[:])
                bcp = ps_o.tile([128, 512], f32, tag="pso", name="bcp")
                nc.tensor.matmul(
                    bcp[0:64, :], lhsT=ones32[:], rhs=rc[:], start=True, stop=True,
                )
                out.append((osb, bcp))
            return out

        def emit_norm_mul_gp(norm, j, b):
            # On GpSimd (otherwise idle): slower per element than DVE, but fully
            # off the DVE/ACT FIFOs, so the broadcast's DMA-queue latency is
            # harmless — nothing else waits on this engine.
            for h01, (osb, bc) in enumerate(norm):
                nc.gpsimd.tensor_mul(
                    ot_all[64 * h01:64 * (h01 + 1), b, j, :],
                    osb[0:64, :],
                    bc[:],
                )

        def emit_norm_mul_dve(norm, j, b):
            # Tail variant: DVE is idle by the last pair; qt-chunked so the
            # first out-projection tiles can start before the full multiply.
            for qt in range(4):
                qsl = slice(qt * 128, (qt + 1) * 128)
                for h01, (osb, bcp) in enumerate(norm):
                    nc.vector.tensor_mul(
                        ot_all[64 * h01:64 * (h01 + 1), b, j, qsl],
                        osb[0:64, qsl],
                        bcp[0:64, qsl],
                    )

        def emit_out_tile(b, qt, nh, pool=None, partial=None, store=None):
            """Output projection tile y[b, qt*128:, nh*512:].

            partial=(ps, lo, hi, finish): continue/finish a held accumulation
            instead of running all 8 pairs at once."""
            if partial is None:
                ps = (pool or ps_big).tile([128, 512], f32, tag=(pool or ps_big).name, name="psy")
                jlo, jhi, finish = 0, NJ, True
            else:
                ps, jlo, jhi, finish = partial
            for jj in range(jlo, jhi):
                nc.tensor.matmul(
                    ps[:],
                    lhsT=ot_all[:, b, jj, qt * 128:(qt + 1) * 128],
                    rhs=wvo_sb[:, jj, nh * 512:(nh + 1) * 512],
                    start=(jj == 0), stop=False,
                )
            if not finish:
                return ps
            nc.tensor.matmul(
                ps[:], lhsT=ones1[:], rhs=bo2_sb[:, nh * 512:(nh + 1) * 512],
                start=False, stop=True,
            )
            ysb = ysb_p.tile([128, 512], f32, tag="ysb", name="ysb")
            nc.vector.tensor_copy(ysb[:], ps[:])
            (store or nc.sync).dma_start(
                out=y[b, qt * 128:(qt + 1) * 128, nh * 512:(nh + 1) * 512],
                in_=ysb[:],
            )
            return None

        # ---- pair 0 projections run right after batch 0's V blocks (the loop
        # emits projections one pair ahead) ----
        qk_tiles = {0: [emit_proj_chain(0, w6) for w6 in range(6)]}

        # per-iteration filler PE work: batch 1's V blocks ride the early
        # iterations, batch 0's out-proj tiles the late ones (batch-major
        # order makes both overlap the pair pipeline instead of serializing)
        extra_sched = {i: [("v", 1, i, nh) for nh in range(2)] for i in range(4)}
        for i in range(7):
            extra_sched[8 + i] = [("o", 0, i // 2, i % 2)]

        pending = None      # (po, es3, j, b) — deferred last-PV + normalize
        mul_pending = None  # (norm, j, b) — deferred GpSimd multiply
        for t in range(NIT):
            b, j = t // NJ, t % NJ
            last = (t == NIT - 1)

            rel = rel_tiles.pop(t)
            qk = qk_tiles.pop(t)
            qk_next = []
            if not last:
                qk_tiles[t + 1] = qk_next
                # 3 projection chains ahead of the score pipeline; the other 3
                # are interleaved between score stages so the PE always has
                # dense independent work while DVE/ACT chew on the softmax.
                for w6 in range(3):
                    qk_next.append(emit_proj_chain(t + 1, w6))

            # kts=0 scores go before the deferred finish: the softmax chain
            # (DVE add -> ACT exp) starts as early as possible
            pss = [ps_s.tile([128, 512], f32, tag="pss", name="pss") for _ in range(2)]
            emit_scores(qk, pss, 0)
            es_prev = emit_softmax(pss, rel, 0)

            # deferred finish of pair t-1: last PV matmul + normalize evac
            if pending is not None:
                ppo, pes, pj, pb = pending
                emit_pv(ppo, pes, pj, pb, 3)
                mul_pending = (emit_norm_head(ppo), pj, pb)
                pending = None

            po = [ps_o.tile([128, 512], f32, tag="pso", name="po") for _ in range(2)]

            if not last:
                for kts in range(1, 4):
                    qk_next.append(emit_proj_chain(t + 1, 2 + kts))
                    pss = [ps_s.tile([128, 512], f32, tag="pss", name="pss") for _ in range(2)]
                    emit_scores(qk, pss, kts)
                    es = emit_softmax(pss, rel, kts)
                    emit_pv(po, es_prev, j, b, kts - 1)
                    es_prev = es
                    if kts == 2:
                        if mul_pending is not None:
                            emit_norm_mul_gp(*mul_pending)
                            mul_pending = None
                        # filler PE work sits mid-iteration so its PSUM
                        # evacuation clears the pool before the next
                        # iteration's projection chains recycle it
                        for task in extra_sched.get(t, ()):
                            if task[0] == "v":
                                emit_v_block(*task[1:])
                            else:
                                pool = ps_big if t % 2 == 0 else ps_s
                                emit_out_tile(*task[1:], pool=pool)
                pending = (po, es_prev, j, b)
                # end-of-body prefetches: anything that might block on a pool
                # buffer must sit behind this iteration's bc broadcasts/stores
                # in its queue, never in front of them
                if t + 2 < NIT:
                    prefetch_rel(t + 2)
                    prefetch_wsl(t + 2)
                if t == 3:
                    nc.sync.dma_start(out=wvo_sb[:], in_=wo[:])
            else:
                # final iteration: no next-pair projections; fill the
                # interleave slots with batch-0's last out tile and batch-1
                # qt=0 partials, then finish this pair inline with the
                # low-latency matmul-broadcast normalize and pipeline the b1
                # out-projection behind it.
                if mul_pending is not None:
                    emit_norm_mul_gp(*mul_pending)
                    mul_pending = None
                part = [None, None]
                for kts in range(1, 4):
                    pss = [ps_s.tile([128, 512], f32, tag="pss", name="pss") for _ in range(2)]
                    emit_scores(qk, pss, kts)
                    es = emit_softmax(pss, rel, kts)
                    emit_pv(po, es_prev, j, b, kts - 1)
                    es_prev = es
                    if kts == 1:
                        emit_out_tile(0, 3, 1)
                    else:
                        nh = kts - 2
                        ps = ps_big.tile([128, 512], f32, tag="psbig", name="psy")
                        part[nh] = emit_out_tile(1, 0, nh, partial=(ps, 0, NJ - 1, False))
                emit_pv(po, es_prev, j, b, 3)
                norm = emit_norm_tail(po)
                emit_norm_mul_dve(norm, j, b)
                # qt=0 tiles: only the last pair's contraction remains; stores
                # go out on the idle GpSimd queue so the sync queue's serial
                # issue cost never tails the kernel.
                for nh in range(2):
                    emit_out_tile(1, 0, nh, partial=(part[nh], NJ - 1, NJ, True),
                                  store=nc.gpsimd)
                for qt in range(1, 4):
                    for nh in range(2):
                        pool = ps_big if nh == 0 else ps_s
                        emit_out_tile(1, qt, nh, pool=pool, store=nc.gpsimd)

    nc.finalize()
    return nc


def prep_inputs(inputs):
    """Host-side sharding + layout prep. Returns per-core in_maps.

    Every device tensor is laid out partition-major so DMAs are linear:
    the value at SBUF (partition p, ...) sits contiguously in DRAM.
    """
    f16 = np.float16
    inputs = {k: np.asarray(v) for k, v in inputs.items()}
    s = float(HD) ** -0.5

    # xt: [4, B, 128p, KT, L] where (kt*128+p) indexes HID of x^T [HID, L]
    xt_full = np.empty((4, B, 128, KT, L), f16)
    for i, k in enumerate(("seq_id", "seq_cate", "seq_pos", "V_id_input")):
        x = inputs[k].astype(f16)                       # [B, L, HID]
        xt = x.transpose(0, 2, 1)                       # [B, HID, L]
        xt_full[i] = xt.reshape(B, KT, 128, L).transpose(0, 2, 1, 3)

    # wqk: [NJ, 128p, 6, KT, 128n] — per head-pair column slices of the six
    # Q/K weight matrices, hid_in = kt*128+p.
    wqk_st = np.stack(
        [inputs[k] for k in ("q_id_w", "k_id_w", "q_cate_w", "k_cate_w", "q_pos_w", "k_pos_w")]
    ).astype(f16)                                       # [6, HID, HID]
    wqk_r = wqk_st.reshape(6, KT, 128, NJ, 128)          # [6, kt, p, j, n]
    wqk_lin = np.ascontiguousarray(wqk_r.transpose(3, 2, 0, 1, 4))  # [j, p, 6, kt, n]

    def w_lin(w):  # [HID, HID] -> [128p, KT, HID]
        return np.ascontiguousarray(
            w.astype(f16).reshape(KT, 128, HID).transpose(1, 0, 2)
        )

    wv_lin = w_lin(inputs["v_id_w"])
    wo_lin = w_lin(inputs["out_w"])

    bqk_st = np.stack(
        [
            inputs["q_id_b"] * s, inputs["k_id_b"],
            inputs["q_cate_b"] * s, inputs["k_cate_b"],
            inputs["q_pos_b"] * s, inputs["k_pos_b"],
        ]
    ).astype(np.float32)                                # [6, HID]
    bqk_lin = np.ascontiguousarray(
        bqk_st.reshape(6, KT, 128).transpose(2, 0, 1)   # [128p, 6, kt]
    ).astype(np.float32)
    # rows of the normalized attention sum to 1, so the V bias collapses into
    # the output bias: y = (A V')Wo + (bv Wo + bo)
    bo2_h = (
        inputs["v_id_b"].astype(np.float64) @ inputs["out_w"].astype(np.float64)
        + inputs["out_b"].astype(np.float64)
    ).astype(f16)

    # relt: [B, NH, 128p, 4kts, L] with (kts*128+p) indexing k of rel^T [k, q]
    relT = np.empty((B, NH, 128, 4, L), f16)
    for b in range(B):
        maskadd = np.where(inputs["attn_mask"][b], np.float32(0), np.float32(MASKVAL))
        relb = inputs["relative_time"][b].astype(np.float32) + maskadd[None]
        rT = relb.transpose(0, 2, 1).astype(f16)         # [NH, k, q]
        relT[b] = rT.reshape(NH, 4, 128, L).transpose(0, 2, 1, 3)

    in_maps = []
    for c in range(NCORES):
        bs = slice(c * BPC, (c + 1) * BPC)
        in_maps.append(
            {
                "xt": np.ascontiguousarray(xt_full[:, bs]),
                "wqk": wqk_lin, "wv": wv_lin, "wo": wo_lin,
                "bqk": bqk_lin, "bo2": bo2_h,
                "relt": np.ascontiguousarray(relT[bs]),
            }
        )
    return in_maps


def kernel(**inputs):
    from concourse.bass_utils import run_bass_kernel_spmd

    if "nc" not in _CACHE:
        _CACHE["nc"] = build_bass()
    nc = _CACHE["nc"]
    in_maps = prep_inputs(inputs)
    res = run_bass_kernel_spmd(nc, in_maps, list(range(NCORES)))
    out = np.concatenate([res.results[c]["y"] for c in range(NCORES)], axis=0)
    return out.astype(np.float32)
